# revision 1
# baseline (speedup 1.0000x reference)
"""AdaptiveMemorySystem kernel: expert-parallel skill MLPs on 8 trn2 NeuronCores.

Sharding: the 50 skill MLPs (Wsk1/Wsk2, ~83% of total FLOPs) are sharded
7-per-core across 8 cores (padded to 56 with zero skills). Each core computes
sum_{s in shard} w_s * (relu(x @ W1_s + b1_s) @ W2_s) over the FULL batch in
bf16 (fp32 accumulation); the 8 partial sums are combined on the host.
Remaining stages (cosine retrieval, top-5 blend, MHA over concepts, fusion)
run in fp32 host-side.
"""

import sys, types
import numpy as np

NUM_CORES = 8
B = D = 1024
SK_PER_CORE = 7  # 8*7 = 56 >= 50, rest zero-padded
NFEAT_TILES = 8  # 1024 / 128
CHUNK = 512
NCHUNK = B // CHUNK

_STATE = {}
LAST_EXEC_NS = None
TRACE = False


def _install_profile_hook():
    try:
        mod = types.ModuleType("antenv.axon_hooks")
        hook_box = [None]
        mod.set_axon_ntff_profile_hook = lambda h: hook_box.__setitem__(0, h)
        mod.get_axon_ntff_profile_hook = lambda: hook_box[0]
        sys.modules.setdefault("antenv.axon_hooks", mod)
        from trn_agent_boot.trn_boot import _ntff_profile_via_ctypes

        if sys.modules["antenv.axon_hooks"] is mod:
            hook_box[0] = _ntff_profile_via_ctypes("/opt/axon/libaxon_pjrt.so")
    except Exception:
        pass


def _build():
    import concourse.bass as bass
    import concourse.bacc as bacc
    import concourse.tile as tile
    import concourse.mybir as mybir

    f32 = mybir.dt.float32
    bf16 = mybir.dt.bfloat16

    nc = bacc.Bacc("TRN2", target_bir_lowering=False, debug=False,
                   num_devices=NUM_CORES)

    xt_ext = nc.dram_tensor("xt", [NFEAT_TILES, 128, B], bf16, kind="ExternalInput")
    w1_ext = nc.dram_tensor("w1", [SK_PER_CORE, NFEAT_TILES, 128, D], bf16,
                            kind="ExternalInput")
    w2_ext = nc.dram_tensor("w2", [SK_PER_CORE, NFEAT_TILES, 128, D], bf16,
                            kind="ExternalInput")
    b1_ext = nc.dram_tensor("b1t", [SK_PER_CORE, 128, NFEAT_TILES], f32,
                            kind="ExternalInput")
    wbc_ext = nc.dram_tensor("wbc", [SK_PER_CORE, 128, B], bf16,
                             kind="ExternalInput")
    out_ext = nc.dram_tensor("proc_out", [D, B], f32, kind="ExternalOutput")

    Relu = mybir.ActivationFunctionType.Relu

    with tile.TileContext(nc) as tc:
        with (
            tc.tile_pool(name="wpool", bufs=2) as wpool,
            tc.tile_pool(name="xpool", bufs=1) as xpool,
            tc.tile_pool(name="hpool", bufs=2) as hpool,
            tc.tile_pool(name="apool", bufs=1) as apool,
            tc.tile_pool(name="p1", bufs=3, space="PSUM") as p1,
            tc.tile_pool(name="p2", bufs=3, space="PSUM") as p2,
        ):
            xt = xpool.tile([128, NFEAT_TILES * B], bf16)
            for k in range(NFEAT_TILES):
                nc.sync.dma_start(xt[:, k * B:(k + 1) * B], xt_ext[k])

            acc = apool.tile([128, NFEAT_TILES * B], f32)

            for s in range(SK_PER_CORE):
                w1t = wpool.tile([128, NFEAT_TILES * D], bf16, tag="w1")
                w2t = wpool.tile([128, NFEAT_TILES * D], bf16, tag="w2")
                b1t = wpool.tile([128, NFEAT_TILES], f32, tag="b1")
                wbt = wpool.tile([128, B], bf16, tag="wb")
                for k in range(NFEAT_TILES):
                    nc.sync.dma_start(w1t[:, k * D:(k + 1) * D], w1_ext[s, k])
                    nc.sync.dma_start(w2t[:, k * D:(k + 1) * D], w2_ext[s, k])
                nc.sync.dma_start(b1t[:], b1_ext[s])
                nc.sync.dma_start(wbt[:], wbc_ext[s])

                for ch in range(NCHUNK):
                    csl = slice(ch * CHUNK, (ch + 1) * CHUNK)
                    hid = hpool.tile([128, NFEAT_TILES * CHUNK], bf16, tag="hid")
                    # layer 1: hid[m] = relu(sum_k W1[k,m].T @ x[k,ch] + b1[m]) * w_s
                    for m in range(NFEAT_TILES):
                        ps = p1.tile([128, CHUNK], f32, tag="ps1")
                        for k in range(NFEAT_TILES):
                            nc.tensor.matmul(
                                ps[:],
                                w1t[:, k * D + m * 128: k * D + (m + 1) * 128],
                                xt[:, k * B + ch * CHUNK: k * B + ch * CHUNK + CHUNK],
                                start=(k == 0), stop=(k == NFEAT_TILES - 1),
                            )
                        hsl = hid[:, m * CHUNK:(m + 1) * CHUNK]
                        nc.scalar.activation(hsl, ps[:], Relu, bias=b1t[:, m:m + 1])
                        nc.vector.tensor_mul(hsl, hsl, wbt[:, csl])
                    # layer 2: acc[m2, ch] += sum_k W2[k,m2].T @ hid[k]
                    for m2 in range(NFEAT_TILES):
                        ps2 = p2.tile([128, CHUNK], f32, tag="ps2")
                        for k in range(NFEAT_TILES):
                            nc.tensor.matmul(
                                ps2[:],
                                w2t[:, k * D + m2 * 128: k * D + (m2 + 1) * 128],
                                hid[:, k * CHUNK:(k + 1) * CHUNK],
                                start=(k == 0), stop=(k == NFEAT_TILES - 1),
                            )
                        asl = acc[:, (m2 * NCHUNK + ch) * CHUNK:
                                  (m2 * NCHUNK + ch + 1) * CHUNK]
                        if s == 0:
                            nc.vector.tensor_copy(asl, ps2[:])
                        else:
                            nc.vector.tensor_add(asl, asl, ps2[:])

            for m2 in range(NFEAT_TILES):
                for ch in range(NCHUNK):
                    nc.sync.dma_start(
                        out_ext[m2 * 128:(m2 + 1) * 128, ch * CHUNK:(ch + 1) * CHUNK],
                        acc[:, (m2 * NCHUNK + ch) * CHUNK:
                            (m2 * NCHUNK + ch + 1) * CHUNK],
                    )

    nc.compile()
    return nc


def _get_nc():
    if "nc" not in _STATE:
        _install_profile_hook()
        _STATE["nc"] = _build()
    return _STATE["nc"]


def _softmax(z):
    z = z - z.max(-1, keepdims=True)
    e = np.exp(z)
    return e / e.sum(-1, keepdims=True)


def _layernorm(h, g, b):
    mu = h.mean(-1, keepdims=True)
    var = h.var(-1, keepdims=True)
    return (h - mu) / np.sqrt(var + 1e-5) * g + b


def _cosine(a, bmat):
    na = np.maximum(np.linalg.norm(a, axis=-1), 1e-8)
    nb = np.maximum(np.linalg.norm(bmat, axis=-1), 1e-8)
    return (a @ bmat.T) / (na[:, None] * nb[None, :])


def kernel(x, working_keys, working_values, working_importance, episode_reprs,
           Wq_wm, bq_wm, concepts, Wq, bq, Wk, bk, Wv, bv, Wo, bo,
           Wk1, bk1, ln1_g, ln1_b, Wk2, bk2, Wsel, bsel,
           Wsk1, bsk1, Wsk2, bsk2, Wf1, bf1, lnf_g, lnf_b, Wf2, bf2):
    global LAST_EXEC_NS
    import ml_dtypes
    from concourse.bass_utils import run_bass_kernel_spmd

    f = np.float32
    x = np.asarray(x, f)
    nc = _get_nc()
    bft = ml_dtypes.bfloat16

    # skill selection weights (host, fp32)
    skill_w = _softmax(x @ np.asarray(Wsel, f) + np.asarray(bsel, f))  # [B,50]

    # per-core shards of the 50 (padded to 56) skills
    xt_b = np.ascontiguousarray(x.T.reshape(NFEAT_TILES, 128, B)).astype(bft)
    S = Wsk1.shape[0]
    in_maps = []
    for c in range(NUM_CORES):
        idx = [i for i in range(c * SK_PER_CORE, (c + 1) * SK_PER_CORE) if i < S]
        n = len(idx)
        w1 = np.zeros((SK_PER_CORE, NFEAT_TILES, 128, D), bft)
        w2 = np.zeros((SK_PER_CORE, NFEAT_TILES, 128, D), bft)
        b1 = np.zeros((SK_PER_CORE, 128, NFEAT_TILES), f)
        wbc = np.zeros((SK_PER_CORE, 128, B), bft)
        if n:
            w1[:n] = np.asarray(Wsk1, f)[idx].reshape(n, NFEAT_TILES, 128, D).astype(bft)
            w2[:n] = np.asarray(Wsk2, f)[idx].reshape(n, NFEAT_TILES, 128, D).astype(bft)
            b1[:n] = np.asarray(bsk1, f)[idx].reshape(n, NFEAT_TILES, 128).transpose(0, 2, 1)
            wbc[:n] = np.broadcast_to(
                skill_w[:, idx].T[:, None, :], (n, 128, B)).astype(bft)
        in_maps.append({"xt": xt_b, "w1": w1, "w2": w2, "b1t": b1, "wbc": wbc})

    res = run_bass_kernel_spmd(nc, in_maps, list(range(NUM_CORES)), trace=TRACE)
    if res.exec_time_ns is not None:
        LAST_EXEC_NS = res.exec_time_ns
    proc_T = np.zeros((D, B), f)
    for r in res.results:
        proc_T += np.asarray(r["proc_out"], f)
    procedural = proc_T.T + skill_w @ np.asarray(bsk2, f)

    # ---- host fp32: working memory (cosine + top-5 softmax blend) ----
    q = x @ np.asarray(Wq_wm, f) + np.asarray(bq_wm, f)
    wm_scores = _cosine(q, np.asarray(working_keys, f)) * np.asarray(
        working_importance, f)[None, :]
    top_i = np.argpartition(-wm_scores, 5, axis=-1)[:, :5]
    top_s = np.take_along_axis(wm_scores, top_i, axis=-1)
    weights = _softmax(top_s)
    working_mem = np.einsum("bk,bkd->bd", weights,
                            np.asarray(working_values, f)[top_i])

    # ---- semantic memory: MHA over concepts + knowledge encoder ----
    H, hd = 8, D // 8
    qh = (x @ np.asarray(Wq, f) + bq).reshape(B, H, hd)
    kh = (np.asarray(concepts, f) @ np.asarray(Wk, f) + bk).reshape(-1, H, hd)
    vh = (np.asarray(concepts, f) @ np.asarray(Wv, f) + bv).reshape(-1, H, hd)
    att = np.einsum("bhd,chd->bhc", qh, kh) / np.sqrt(np.float32(hd))
    att = _softmax(att)
    attended = np.einsum("bhc,chd->bhd", att, vh).reshape(B, D) @ np.asarray(Wo, f) + bo
    combined = x + attended
    semantic = np.maximum(
        _layernorm(combined @ np.asarray(Wk1, f) + bk1, ln1_g, ln1_b), 0.0
    ) @ np.asarray(Wk2, f) + bk2

    # ---- episodic: best cosine episode ----
    ep = np.asarray(episode_reprs, f)
    episodic = ep[np.argmax(_cosine(x, ep), axis=-1)]

    # ---- fusion ----
    all_mem = np.concatenate([working_mem, episodic, semantic, procedural], axis=-1)
    fused = np.maximum(
        _layernorm(all_mem @ np.asarray(Wf1, f) + bf1, lnf_g, lnf_b), 0.0
    ) @ np.asarray(Wf2, f) + bf2
    return fused.astype(np.float32)


# revision 2
# speedup vs baseline: 1.2024x; 1.2024x over previous
"""AdaptiveMemorySystem kernel: expert-parallel skill MLPs on 8 trn2 NeuronCores.

Sharding: the 50 skill MLPs (Wsk1/Wsk2, ~83% of total FLOPs) are sharded
7-per-core across 8 cores (padded to 56 with zero skills). Each core computes
sum_{s in shard} w_s * (relu(x @ W1_s + b1_s) @ W2_s) over the FULL batch in
bf16 (fp32 accumulation); the 8 partial sums are combined on the host.
Remaining stages (cosine retrieval, top-5 blend, MHA over concepts, fusion)
run in fp32 host-side.
"""

import sys, types
import numpy as np

NUM_CORES = 8
B = D = 1024
SK_PER_CORE = 7  # 8*7 = 56 >= 50, rest zero-padded
NFEAT_TILES = 8  # 1024 / 128
CHUNK = 512
NCHUNK = B // CHUNK

_STATE = {}
LAST_EXEC_NS = None
TRACE = False


def _install_profile_hook():
    try:
        mod = types.ModuleType("antenv.axon_hooks")
        hook_box = [None]
        mod.set_axon_ntff_profile_hook = lambda h: hook_box.__setitem__(0, h)
        mod.get_axon_ntff_profile_hook = lambda: hook_box[0]
        sys.modules.setdefault("antenv.axon_hooks", mod)
        from trn_agent_boot.trn_boot import _ntff_profile_via_ctypes

        if sys.modules["antenv.axon_hooks"] is mod:
            hook_box[0] = _ntff_profile_via_ctypes("/opt/axon/libaxon_pjrt.so")
    except Exception:
        pass


def _build():
    import concourse.bass as bass
    import concourse.bacc as bacc
    import concourse.tile as tile
    import concourse.mybir as mybir

    f32 = mybir.dt.float32
    bf16 = mybir.dt.bfloat16

    nc = bacc.Bacc("TRN2", target_bir_lowering=False, debug=False,
                   num_devices=NUM_CORES)

    xt_ext = nc.dram_tensor("xt", [NFEAT_TILES, 128, B], bf16, kind="ExternalInput")
    w1_ext = nc.dram_tensor("w1", [SK_PER_CORE, NFEAT_TILES, 128, D], bf16,
                            kind="ExternalInput")
    w2_ext = nc.dram_tensor("w2", [SK_PER_CORE, NFEAT_TILES, 128, D], bf16,
                            kind="ExternalInput")
    b1_ext = nc.dram_tensor("b1t", [SK_PER_CORE, 128, NFEAT_TILES], f32,
                            kind="ExternalInput")
    wbc_ext = nc.dram_tensor("wbc", [SK_PER_CORE, 128, B], bf16,
                             kind="ExternalInput")
    out_ext = nc.dram_tensor("proc_out", [D, B], f32, kind="ExternalOutput")

    Relu = mybir.ActivationFunctionType.Relu

    with tile.TileContext(nc) as tc:
        with (
            tc.tile_pool(name="wpool", bufs=2) as wpool,
            tc.tile_pool(name="xpool", bufs=1) as xpool,
            tc.tile_pool(name="hpool", bufs=3) as hpool,
            tc.tile_pool(name="apool", bufs=1) as apool,
            tc.tile_pool(name="p1", bufs=4, space="PSUM") as p1,
            tc.tile_pool(name="p2", bufs=3, space="PSUM") as p2,
        ):
            xt = xpool.tile([128, NFEAT_TILES * B], bf16)
            for k in range(NFEAT_TILES):
                nc.sync.dma_start(xt[:, k * B:(k + 1) * B], xt_ext[k])

            acc = apool.tile([128, NFEAT_TILES * B], f32)

            for s in range(SK_PER_CORE):
                w1t = wpool.tile([128, NFEAT_TILES * D], bf16, tag="w1")
                w2t = wpool.tile([128, NFEAT_TILES * D], bf16, tag="w2")
                b1t = wpool.tile([128, NFEAT_TILES], f32, tag="b1")
                wbt = wpool.tile([128, B], bf16, tag="wb")
                for k in range(NFEAT_TILES):
                    nc.sync.dma_start(w1t[:, k * D:(k + 1) * D], w1_ext[s, k])
                    nc.sync.dma_start(w2t[:, k * D:(k + 1) * D], w2_ext[s, k])
                nc.sync.dma_start(b1t[:], b1_ext[s])
                nc.sync.dma_start(wbt[:], wbc_ext[s])

                for ch in range(NCHUNK):
                    csl = slice(ch * CHUNK, (ch + 1) * CHUNK)
                    hid = hpool.tile([128, NFEAT_TILES * CHUNK], bf16, tag="hid")
                    # layer 1: hid[m] = relu(sum_k W1[k,m].T @ x[k,ch] + b1[m]) * w_s
                    for m in range(NFEAT_TILES):
                        ps = p1.tile([128, CHUNK], f32, tag="ps1")
                        for k in range(NFEAT_TILES):
                            nc.tensor.matmul(
                                ps[:],
                                w1t[:, k * D + m * 128: k * D + (m + 1) * 128],
                                xt[:, k * B + ch * CHUNK: k * B + ch * CHUNK + CHUNK],
                                start=(k == 0), stop=(k == NFEAT_TILES - 1),
                            )
                        hsl = hid[:, m * CHUNK:(m + 1) * CHUNK]
                        nc.scalar.activation(hsl, ps[:], Relu, bias=b1t[:, m:m + 1])
                        nc.vector.tensor_mul(hsl, hsl, wbt[:, csl])
                    # layer 2: acc[m2, ch] += sum_k W2[k,m2].T @ hid[k]
                    for m2 in range(NFEAT_TILES):
                        ps2 = p2.tile([128, CHUNK], f32, tag="ps2")
                        for k in range(NFEAT_TILES):
                            nc.tensor.matmul(
                                ps2[:],
                                w2t[:, k * D + m2 * 128: k * D + (m2 + 1) * 128],
                                hid[:, k * CHUNK:(k + 1) * CHUNK],
                                start=(k == 0), stop=(k == NFEAT_TILES - 1),
                            )
                        asl = acc[:, (m2 * NCHUNK + ch) * CHUNK:
                                  (m2 * NCHUNK + ch + 1) * CHUNK]
                        if s == 0:
                            nc.vector.tensor_copy(asl, ps2[:])
                        else:
                            nc.vector.tensor_add(asl, asl, ps2[:])

            for m2 in range(NFEAT_TILES):
                for ch in range(NCHUNK):
                    nc.sync.dma_start(
                        out_ext[m2 * 128:(m2 + 1) * 128, ch * CHUNK:(ch + 1) * CHUNK],
                        acc[:, (m2 * NCHUNK + ch) * CHUNK:
                            (m2 * NCHUNK + ch + 1) * CHUNK],
                    )

    nc.compile()
    return nc


def _get_nc():
    if "nc" not in _STATE:
        _install_profile_hook()
        _STATE["nc"] = _build()
    return _STATE["nc"]


def _softmax(z):
    z = z - z.max(-1, keepdims=True)
    e = np.exp(z)
    return e / e.sum(-1, keepdims=True)


def _layernorm(h, g, b):
    mu = h.mean(-1, keepdims=True)
    var = h.var(-1, keepdims=True)
    return (h - mu) / np.sqrt(var + 1e-5) * g + b


def _cosine(a, bmat):
    na = np.maximum(np.linalg.norm(a, axis=-1), 1e-8)
    nb = np.maximum(np.linalg.norm(bmat, axis=-1), 1e-8)
    return (a @ bmat.T) / (na[:, None] * nb[None, :])


def kernel(x, working_keys, working_values, working_importance, episode_reprs,
           Wq_wm, bq_wm, concepts, Wq, bq, Wk, bk, Wv, bv, Wo, bo,
           Wk1, bk1, ln1_g, ln1_b, Wk2, bk2, Wsel, bsel,
           Wsk1, bsk1, Wsk2, bsk2, Wf1, bf1, lnf_g, lnf_b, Wf2, bf2):
    global LAST_EXEC_NS
    import ml_dtypes
    from concourse.bass_utils import run_bass_kernel_spmd

    f = np.float32
    x = np.asarray(x, f)
    nc = _get_nc()
    bft = ml_dtypes.bfloat16

    # skill selection weights (host, fp32)
    skill_w = _softmax(x @ np.asarray(Wsel, f) + np.asarray(bsel, f))  # [B,50]

    # per-core shards of the 50 (padded to 56) skills
    xt_b = np.ascontiguousarray(x.T.reshape(NFEAT_TILES, 128, B)).astype(bft)
    S = Wsk1.shape[0]
    in_maps = []
    for c in range(NUM_CORES):
        idx = [i for i in range(c * SK_PER_CORE, (c + 1) * SK_PER_CORE) if i < S]
        n = len(idx)
        w1 = np.zeros((SK_PER_CORE, NFEAT_TILES, 128, D), bft)
        w2 = np.zeros((SK_PER_CORE, NFEAT_TILES, 128, D), bft)
        b1 = np.zeros((SK_PER_CORE, 128, NFEAT_TILES), f)
        wbc = np.zeros((SK_PER_CORE, 128, B), bft)
        if n:
            w1[:n] = np.asarray(Wsk1, f)[idx].reshape(n, NFEAT_TILES, 128, D).astype(bft)
            w2[:n] = np.asarray(Wsk2, f)[idx].reshape(n, NFEAT_TILES, 128, D).astype(bft)
            b1[:n] = np.asarray(bsk1, f)[idx].reshape(n, NFEAT_TILES, 128).transpose(0, 2, 1)
            wbc[:n] = np.broadcast_to(
                skill_w[:, idx].T[:, None, :], (n, 128, B)).astype(bft)
        in_maps.append({"xt": xt_b, "w1": w1, "w2": w2, "b1t": b1, "wbc": wbc})

    res = run_bass_kernel_spmd(nc, in_maps, list(range(NUM_CORES)), trace=TRACE)
    if res.exec_time_ns is not None:
        LAST_EXEC_NS = res.exec_time_ns
    proc_T = np.zeros((D, B), f)
    for r in res.results:
        proc_T += np.asarray(r["proc_out"], f)
    procedural = proc_T.T + skill_w @ np.asarray(bsk2, f)

    # ---- host fp32: working memory (cosine + top-5 softmax blend) ----
    q = x @ np.asarray(Wq_wm, f) + np.asarray(bq_wm, f)
    wm_scores = _cosine(q, np.asarray(working_keys, f)) * np.asarray(
        working_importance, f)[None, :]
    top_i = np.argpartition(-wm_scores, 5, axis=-1)[:, :5]
    top_s = np.take_along_axis(wm_scores, top_i, axis=-1)
    weights = _softmax(top_s)
    working_mem = np.einsum("bk,bkd->bd", weights,
                            np.asarray(working_values, f)[top_i])

    # ---- semantic memory: MHA over concepts + knowledge encoder ----
    H, hd = 8, D // 8
    qh = (x @ np.asarray(Wq, f) + bq).reshape(B, H, hd)
    kh = (np.asarray(concepts, f) @ np.asarray(Wk, f) + bk).reshape(-1, H, hd)
    vh = (np.asarray(concepts, f) @ np.asarray(Wv, f) + bv).reshape(-1, H, hd)
    att = np.einsum("bhd,chd->bhc", qh, kh) / np.sqrt(np.float32(hd))
    att = _softmax(att)
    attended = np.einsum("bhc,chd->bhd", att, vh).reshape(B, D) @ np.asarray(Wo, f) + bo
    combined = x + attended
    semantic = np.maximum(
        _layernorm(combined @ np.asarray(Wk1, f) + bk1, ln1_g, ln1_b), 0.0
    ) @ np.asarray(Wk2, f) + bk2

    # ---- episodic: best cosine episode ----
    ep = np.asarray(episode_reprs, f)
    episodic = ep[np.argmax(_cosine(x, ep), axis=-1)]

    # ---- fusion ----
    all_mem = np.concatenate([working_mem, episodic, semantic, procedural], axis=-1)
    fused = np.maximum(
        _layernorm(all_mem @ np.asarray(Wf1, f) + bf1, lnf_g, lnf_b), 0.0
    ) @ np.asarray(Wf2, f) + bf2
    return fused.astype(np.float32)
